# revision 1
# baseline (speedup 1.0000x reference)
"""Trainium2 Bass kernel for the 2-layer ConcatLSTM problem.

Sharding: data-parallel over batch (B=64 -> 8 per core), weights replicated.

Device layout notes (per core, batch BC=8):
- Gate tile in PSUM: [128 partitions, 512 free]. Partition p = 32*j + b where
  j = H-slice (0..3) and b = batch (0..7); rows 32j+8..32j+31 are junk.
  Free f = gp*128 + h' where gp = gate position in order (i, f, o, g) and
  h = j*128 + h' is the hidden index.
- Recurrent matmul: h-stationary, 4 K-chunks x 4 col-groups (tile_position
  col tiling) streaming W_hh^T (permuted) as the moving operand.
- Cell state c and activated gates live in the same [32j+b, h'] layout, so
  every elementwise op is a plain FD-sliced op on aligned partitions.
- h is transposed once per step on the PE (via identity matmul) to produce
  hT [h', (j, b)] which feeds the next step's stationary operand and the
  layer-1 input projection.
"""

import os
import sys

sys.path.insert(0, "/opt/trn_rl_repo")

import numpy as np
from contextlib import ExitStack

from concourse import bacc, tile, mybir
from concourse.bass_utils import run_bass_kernel_spmd
from concourse.masks import make_identity

T, B, I_IN, H, C = 512, 64, 256, 512, 64
G = 4 * H
NCORES = 8
BC = B // NCORES        # batch per core
K0 = 384                # 256 + 64 + 1 padded to 3*128
K1 = 640                # 512 + 64 + 1 padded to 5*128
F32 = mybir.dt.float32
AF = mybir.ActivationFunctionType

# device gate order (i, f, o, g); reference splits gates as (i, f, g, o)
ORIG_GATE = (0, 1, 3, 2)


def _gate_perm() -> np.ndarray:
    perm = np.empty(G, np.int64)
    for j in range(4):
        for gp, og in enumerate(ORIG_GATE):
            src = og * 512 + j * 128
            dst = j * 512 + gp * 128
            perm[dst:dst + 128] = np.arange(src, src + 128)
    return perm


PERM = _gate_perm()


def _xw_phase(tc, ctx, nc, name, lhsT_of, k_chunks, w_dram, xw_dram, mtiles):
    """xw[(t,b), :] = lhsT.T @ W  accumulated over k chunks; DMA PSUM->DRAM."""
    wpool = ctx.enter_context(tc.tile_pool(name=f"{name}_w", bufs=1))
    wt = []
    for k in range(k_chunks):
        w = wpool.tile([128, G], F32, tag=f"w{k}")
        nc.sync.dma_start(w[:], w_dram[k * 128:(k + 1) * 128, :])
        wt.append(w)
    pp = ctx.enter_context(tc.tile_pool(name=f"{name}_ps", bufs=2, space="PSUM"))
    sg = ctx.enter_context(tc.tile_pool(name=f"{name}_sg", bufs=2))
    for m in range(mtiles):
        ps = pp.tile([128, G], F32)
        for n in range(4):
            for k in range(k_chunks):
                nc.tensor.matmul(
                    ps[:, n * 512:(n + 1) * 512],
                    lhsT_of(m, k),
                    wt[k][:, n * 512:(n + 1) * 512],
                    start=(k == 0),
                    stop=(k == k_chunks - 1),
                )
        stg = sg.tile([128, G], F32)
        nc.scalar.copy(stg[:], ps[:])
        nc.sync.dma_start(xw_dram[m * 128:(m + 1) * 128, :], stg[:])


def _recur_phase(tc, ctx, nc, name, wh_dram, xw_dram, t_steps, ident,
                 hist=None, ring=None, y_out=None):
    """One LSTM layer recurrence over t_steps.

    hist: SBUF AP [128, t_steps*32] to store compact hT history (layer 0), or
    ring: SBUF AP [128, 64] two-slot hT ring (layer 1).
    y_out: DRAM AP [t_steps, BC, H] to store h per step (layer 1).
    """
    whp = ctx.enter_context(tc.tile_pool(name=f"{name}_wh", bufs=1))
    wh = []
    for k in range(4):
        w = whp.tile([128, G], F32, tag=f"wh{k}")
        nc.sync.dma_start(w[:], wh_dram[k * 128:(k + 1) * 128, :])
        wh.append(w)

    st = ctx.enter_context(tc.tile_pool(name=f"{name}_st", bufs=1))
    c_t = st.tile([128, 128], F32)
    nc.gpsimd.memset(c_t[:], 0.0)
    zer = st.tile([128, 8], F32)
    nc.gpsimd.memset(zer[:], 0.0)

    xwp = ctx.enter_context(tc.tile_pool(name=f"{name}_xw", bufs=6))
    gp = ctx.enter_context(tc.tile_pool(name=f"{name}_g", bufs=2, space="PSUM"))
    tp = ctx.enter_context(tc.tile_pool(name=f"{name}_tp", bufs=2, space="PSUM"))
    sp = ctx.enter_context(tc.tile_pool(name=f"{name}_s", bufs=3))
    hp = ctx.enter_context(tc.tile_pool(name=f"{name}_h", bufs=3))

    # hT storage layout: 4 planes (one per k-chunk of H), each plane holds
    # [t*8 + b] compactly: FD = k*plane + t*8 + b. This keeps every matmul
    # stationary-operand slice single-strided (walrus requires 1 free dim).
    if hist is not None:
        plane = t_steps * 8
        store = hist
        n_slots = t_steps
    else:
        plane = 16
        store = ring
        n_slots = 2
    store4 = store.rearrange("p (j f) -> p j f", j=4)

    def hT_write_dst(t):
        sl = t % n_slots
        return store4[:, :, sl * 8:(sl + 1) * 8]

    def hT_read(t, k):
        sl = t % n_slots
        return store[:, k * plane + sl * 8: k * plane + (sl + 1) * 8]

    for t in range(t_steps):
        # prefetch xw_t into [32j+b, (gp,h')] layout (one DMA per j: SBUF
        # partition dims cannot be split by rearrange)
        xt = xwp.tile([128, 512], F32)
        for j in range(4):
            nc.sync.dma_start(
                xt[32 * j:32 * j + BC, :],
                xw_dram[t * BC:(t + 1) * BC, j * 512:(j + 1) * 512])

        ps = gp.tile([128, 512], F32)
        for k in range(4):
            if t == 0:
                lh = zer[:]
            else:
                lh = hT_read(t - 1, k)
            for j in range(4):
                nc.tensor.matmul(
                    ps[32 * j:32 * j + BC, :],
                    lh,
                    wh[k][:, j * 512:(j + 1) * 512],
                    start=(k == 0),
                    stop=(k == 3),
                    tile_position=(0, 32 * j),
                )

        # gates += xw_t
        nc.vector.tensor_add(ps[:], ps[:], xt[:])

        # activations: sigmoid over (i, f, o), tanh over g
        s = sp.tile([128, 512], F32)
        nc.scalar.activation(s[:, 0:384], ps[:, 0:384], AF.Sigmoid)
        nc.scalar.activation(s[:, 384:512], ps[:, 384:512], AF.Tanh)

        # c = f*c + i*g ; h = o * tanh(c)
        ig = hp.tile([128, 128], F32, tag="ig")
        nc.vector.tensor_mul(ig[:], s[:, 0:128], s[:, 384:512])
        fc = hp.tile([128, 128], F32, tag="fc")
        nc.vector.tensor_mul(fc[:], s[:, 128:256], c_t[:])
        nc.vector.tensor_add(c_t[:], fc[:], ig[:])
        th = hp.tile([128, 128], F32, tag="th")
        nc.scalar.activation(th[:], c_t[:], AF.Tanh)
        h = hp.tile([128, 128], F32, tag="h")
        nc.vector.tensor_mul(h[:], s[:, 256:384], th[:])

        # transpose h -> hT and compact into the hist/ring slot
        pt = tp.tile([128, 128], F32)
        nc.tensor.transpose(pt[:], h[:], ident[:])
        src_hT = pt.rearrange("p (j r) -> p j r", r=32)[:, :, 0:BC]
        nc.vector.tensor_copy(hT_write_dst(t), src_hT)

        if y_out is not None:
            for j in range(4):
                nc.sync.dma_start(
                    y_out[t * BC:(t + 1) * BC, j * 128:(j + 1) * 128],
                    h[32 * j:32 * j + BC, :])


def build_nc(t_steps=T):
    nc = bacc.Bacc("TRN2", target_bir_lowering=False, debug=False,
                   enable_asserts=False, num_devices=NCORES)
    mt = t_steps * BC // 128

    xT = nc.dram_tensor("xT", [K0, t_steps * BC], F32, kind="ExternalInput").ap()
    w0 = nc.dram_tensor("w0", [K0, G], F32, kind="ExternalInput").ap()
    wh0 = nc.dram_tensor("wh0", [H, G], F32, kind="ExternalInput").ap()
    w1 = nc.dram_tensor("w1", [K1, G], F32, kind="ExternalInput").ap()
    wh1 = nc.dram_tensor("wh1", [H, G], F32, kind="ExternalInput").ap()
    aug4 = nc.dram_tensor("aug4", [128, 128], F32, kind="ExternalInput").ap()
    y = nc.dram_tensor("y", [t_steps * BC, H], F32, kind="ExternalOutput").ap()
    xw0d = nc.dram_tensor("xw0d", [t_steps * BC, G], F32, kind="Internal").ap()
    xw1d = nc.dram_tensor("xw1d", [t_steps * BC, G], F32, kind="Internal").ap()

    with tile.TileContext(nc) as tc:
        with ExitStack() as octx:
            misc = octx.enter_context(tc.tile_pool(name="misc", bufs=1))
            ident = misc.tile([128, 128], F32)
            make_identity(nc, ident[:])
            aug4_sb = misc.tile([128, 128], F32)
            nc.sync.dma_start(aug4_sb[:], aug4[:])
            hist = misc.tile([128, t_steps * 32], F32)

            # phase A: xw0 = [x | ctx | 1] @ [W_ih0 | W_mh0 | b0] (permuted)
            with ExitStack() as c1:
                xp = c1.enter_context(tc.tile_pool(name="a_x", bufs=1))
                xts = []
                for k in range(3):
                    xt_ = xp.tile([128, t_steps * BC], F32, tag=f"x{k}")
                    nc.sync.dma_start(xt_[:], xT[k * 128:(k + 1) * 128, :])
                    xts.append(xt_)
                _xw_phase(tc, c1, nc, "a",
                          lambda m, k: xts[k][:, m * 128:(m + 1) * 128],
                          3, w0, xw0d, mt)

            # phase B: layer-0 recurrence -> hist
            with ExitStack() as c2:
                _recur_phase(tc, c2, nc, "b", wh0, xw0d, t_steps, ident,
                             hist=hist)

            # phase C: xw1 = [y0 | ctx | 1] @ [W_ih1 | W_mh1 | b1]
            with ExitStack() as c3:
                plane = t_steps * 8

                def lhsT1(m, k):
                    if k == 4:
                        return aug4_sb[:]
                    return hist[:, k * plane + m * 128: k * plane + (m + 1) * 128]

                _xw_phase(tc, c3, nc, "c", lhsT1, 5, w1, xw1d, mt)

            # phase D: layer-1 recurrence -> y
            with ExitStack() as c4:
                rp = c4.enter_context(tc.tile_pool(name="d_r", bufs=1))
                ring = rp.tile([128, 64], F32)
                _recur_phase(tc, c4, nc, "d", wh1, xw1d, t_steps, ident,
                             ring=ring, y_out=y)

    nc.finalize()
    return nc


def host_inputs(x, date_contexts, w_ih0, w_hh0, w_mh0, b0,
                w_ih1, w_hh1, w_mh1, b1, t_steps=T):
    """Build per-core input maps (layout prep only, no heavy math)."""
    f = lambda a: np.ascontiguousarray(np.asarray(a, np.float32))
    x, ctx = f(x), f(date_contexts)
    w0aug = np.zeros((K0, G), np.float32)
    w0aug[0:I_IN] = f(w_ih0).T
    w0aug[I_IN:I_IN + C] = f(w_mh0).T
    w0aug[I_IN + C] = f(b0)
    w0aug = np.ascontiguousarray(w0aug[:, PERM])
    wh0p = np.ascontiguousarray(f(w_hh0).T[:, PERM])
    w1aug = np.zeros((K1, G), np.float32)
    w1aug[0:H] = f(w_ih1).T
    w1aug[H:H + C] = f(w_mh1).T
    w1aug[H + C] = f(b1)
    w1aug = np.ascontiguousarray(w1aug[:, PERM])
    wh1p = np.ascontiguousarray(f(w_hh1).T[:, PERM])

    in_maps = []
    for c in range(NCORES):
        bs = slice(c * BC, (c + 1) * BC)
        xTc = np.zeros((K0, t_steps, BC), np.float32)
        xTc[0:I_IN] = np.moveaxis(x[:t_steps, bs, :], 2, 0)
        xTc[I_IN:I_IN + C] = ctx[bs].T[:, None, :]
        xTc[I_IN + C] = 1.0
        a4 = np.zeros((128, 128), np.float32)
        a4[0:C] = np.broadcast_to(
            ctx[bs].T[:, None, :], (C, 16, BC)).reshape(C, 128)
        a4[C] = 1.0
        in_maps.append({
            "xT": np.ascontiguousarray(xTc.reshape(K0, t_steps * BC)),
            "w0": w0aug, "wh0": wh0p, "w1": w1aug, "wh1": wh1p, "aug4": a4,
        })
    return in_maps


_NC_CACHE = {}


def _get_nc(t_steps=T):
    if t_steps not in _NC_CACHE:
        _NC_CACHE[t_steps] = build_nc(t_steps)
    return _NC_CACHE[t_steps]


def kernel(x, date_contexts, w_ih0, w_hh0, w_mh0, b0,
           w_ih1, w_hh1, w_mh1, b1):
    t_steps = int(os.environ.get("LSTM_T_STEPS", T))
    in_maps = host_inputs(x, date_contexts, w_ih0, w_hh0, w_mh0, b0,
                          w_ih1, w_hh1, w_mh1, b1, t_steps)
    nc = _get_nc(t_steps)
    res = run_bass_kernel_spmd(nc, in_maps, core_ids=list(range(NCORES)))
    out = np.stack(
        [res.results[c]["y"].reshape(t_steps, BC, H) for c in range(NCORES)],
        axis=1,
    )  # [T, NCORES, BC, H]
    return np.ascontiguousarray(
        out.reshape(t_steps, B, H)).astype(np.float32)



# revision 9
# speedup vs baseline: 1.0408x; 1.0408x over previous
"""Trainium2 Bass kernel for the 2-layer ConcatLSTM problem (v2).

Sharding: data-parallel over batch (B=64 -> 8 per core), weights replicated.

v2 design (vs v1: fp32, 4 serial phases):
- All matmul operands bf16 (PSUM accumulation stays fp32): 4x fewer PE
  cycles per moving row.
- Single fused main loop: layer-0 step t and layer-1 step t-32 run
  back-to-back on the PE each iteration, so each layer's elementwise tail
  (activations, cell update, transpose) hides under the other layer's
  matmul stream instead of stalling the PE.
- xw projections (x@W_ih + ctx@W_mh + b) are computed inline in 16-step
  chunks (one [128,2048] tile per chunk), spread across the window's
  iterations, instead of as separate phases; results round-trip DRAM in
  bf16 only to get the [row, G] -> [32j+b, 512] partition swizzle for free.
- Stationary h^T slots are 32 columns wide (8 real + 24 junk) so the
  recurrent matmuls write all 128 PSUM partitions; every tile is fully
  written before being read (keeps CoreSim usable; junk lanes never reach
  real outputs).

Device layout (per core, batch BC=8), unchanged from v1:
- Gate tile in PSUM: [128, 512]; partition p = 32*j + b (j = H-slice,
  b = batch; rows 32j+8..32j+31 are junk). Free f = gp*128 + h' with gate
  order (i, f, o, g) and h = j*128 + h'.
- Recurrent matmul: h-stationary [128, 32] slices, 4 K-chunks x 4 col
  groups streaming W_hh^T (permuted, bf16).
"""

import os
import sys

sys.path.insert(0, "/opt/trn_rl_repo")

import numpy as np
import ml_dtypes
from contextlib import ExitStack

from concourse import bacc, tile, mybir
from concourse.bass_utils import run_bass_kernel_spmd
from concourse.masks import make_identity

T, B, I_IN, H, C = 512, 64, 256, 512, 64
G = 4 * H
NCORES = 8
BC = B // NCORES        # batch per core
K0 = 384                # 256 + 64 + 1 padded to 3*128
K1 = 640                # 512 + 64 + 1 padded to 5*128
F32 = mybir.dt.float32
BF16 = mybir.dt.bfloat16
AF = mybir.ActivationFunctionType
BF16NP = ml_dtypes.bfloat16

LAG = 32                # layer-1 runs LAG steps behind layer-0
CH = 16                 # xw projection chunk = CH steps (CH*BC = 128 rows)

# device gate order (i, f, o, g); reference splits gates as (i, f, g, o)
ORIG_GATE = (0, 1, 3, 2)


def _gate_perm() -> np.ndarray:
    perm = np.empty(G, np.int64)
    for j in range(4):
        for gp, og in enumerate(ORIG_GATE):
            src = og * 512 + j * 128
            dst = j * 512 + gp * 128
            perm[dst:dst + 128] = np.arange(src, src + 128)
    return perm


PERM = _gate_perm()


def build_nc(t_steps=T):
    nc = bacc.Bacc("TRN2", target_bir_lowering=False, debug=False,
                   enable_asserts=False, num_devices=NCORES)
    n_ch = t_steps // CH
    plane = t_steps * 8          # hist columns per K-chunk

    xT = nc.dram_tensor("xT", [K0, t_steps * BC], BF16, kind="ExternalInput").ap()
    w0 = nc.dram_tensor("w0", [K0, G], BF16, kind="ExternalInput").ap()
    wh0 = nc.dram_tensor("wh0", [H, G], BF16, kind="ExternalInput").ap()
    w1 = nc.dram_tensor("w1", [K1, G], BF16, kind="ExternalInput").ap()
    wh1 = nc.dram_tensor("wh1", [H, G], BF16, kind="ExternalInput").ap()
    aug4 = nc.dram_tensor("aug4", [128, 128], BF16, kind="ExternalInput").ap()
    y = nc.dram_tensor("y", [t_steps * BC, H], F32, kind="ExternalOutput").ap()
    # +24 pad rows: per-step xt loads read 32 rows to cover junk partitions
    xw0d = nc.dram_tensor("xw0d", [t_steps * BC + 24, G], BF16, kind="Internal").ap()
    xw1d = nc.dram_tensor("xw1d", [t_steps * BC + 24, G], BF16, kind="Internal").ap()

    with tile.TileContext(nc) as tc:
      with ExitStack() as ctx:
        misc = ctx.enter_context(tc.tile_pool(name="misc", bufs=1))
        ident = misc.tile([128, 128], F32)
        make_identity(nc, ident[:])
        aug4_sb = misc.tile([128, 128], BF16)
        nc.sync.dma_start(aug4_sb[:], aug4[:])
        # hist: hT history of layer 0; slot (k, t) at col k*plane + t*8,
        # stationary reads are 32 wide -> pad 24 cols, memset once.
        hist = misc.tile([128, 4 * plane + 24], BF16)
        nc.gpsimd.memset(hist[:, 4 * plane:], 0.0)
        # ring: layer-1 hT, 2 slots of 32 per K-chunk: col k*64 + sl*32
        ring = misc.tile([128, 256], BF16)
        zer = misc.tile([128, 32], BF16)
        nc.gpsimd.memset(zer[:], 0.0)

        # resident x^T (bf16): 3 K-chunks of [128, t*8]
        xts = []
        for k in range(3):
            t_ = misc.tile([128, t_steps * BC], BF16, tag=f"xT{k}")
            nc.sync.dma_start(t_[:], xT[k * 128:(k + 1) * 128, :])
            xts.append(t_)

        # resident weights (bf16)
        def load_w(name, dram, kchunks):
            out = []
            for k in range(kchunks):
                w = misc.tile([128, G], BF16, tag=f"{name}{k}")
                nc.sync.dma_start(w[:], dram[k * 128:(k + 1) * 128, :])
                out.append(w)
            return out

        w0t = load_w("w0", w0, 3)
        wh0t = load_w("wh0", wh0, 4)
        w1t = load_w("w1", w1, 5)
        wh1t = load_w("wh1", wh1, 4)

        # cell state + per-step pools
        st = ctx.enter_context(tc.tile_pool(name="st", bufs=1))
        c0_t = st.tile([128, 128], F32, tag="c0")
        nc.gpsimd.memset(c0_t[:], 0.0)
        c1_t = st.tile([128, 128], F32, tag="c1")
        nc.gpsimd.memset(c1_t[:], 0.0)

        xwp = ctx.enter_context(tc.tile_pool(name="xw", bufs=12))
        gp0 = ctx.enter_context(tc.tile_pool(name="g0", bufs=2, space="PSUM"))
        gp1 = ctx.enter_context(tc.tile_pool(name="g1", bufs=2, space="PSUM"))
        tp = ctx.enter_context(tc.tile_pool(name="tp", bufs=2, space="PSUM"))
        pp = ctx.enter_context(tc.tile_pool(name="pj", bufs=2, space="PSUM"))
        sp = ctx.enter_context(tc.tile_pool(name="s", bufs=4))
        hp = ctx.enter_context(tc.tile_pool(name="h", bufs=6))
        pjs = ctx.enter_context(tc.tile_pool(name="pjs", bufs=2))

        # ---- helpers ------------------------------------------------------
        def proj_piece(layer, chunk, piece):
            """One n-quadrant of an xw chunk projection.

            layer 0: xw0[chunk] rows = steps [chunk*CH, ..), K=3 chunks of xT
            layer 1: xw1[chunk] from hist cols chunk*128 (+aug4), K=5
            piece = n in 0..3 selects G columns [n*512, (n+1)*512).
            """
            n = piece
            if layer == 0:
                kch, wt = 3, w0t
                lhs = lambda k: xts[k][:, chunk * 128:(chunk + 1) * 128]
                dst = xw0d
            else:
                kch, wt = 5, w1t
                lhs = lambda k: (aug4_sb[:] if k == 4 else
                                 hist[:, k * plane + chunk * 128:
                                      k * plane + chunk * 128 + 128])
                dst = xw1d
            ps = pp.tile([128, 512], F32)
            for k in range(kch):
                nc.tensor.matmul(ps[:], lhs(k), wt[k][:, n * 512:(n + 1) * 512],
                                 start=(k == 0), stop=(k == kch - 1))
            stg = pjs.tile([128, 512], BF16)
            nc.scalar.copy(stg[:], ps[:])
            nc.sync.dma_start(
                dst[chunk * 128:(chunk + 1) * 128, n * 512:(n + 1) * 512],
                stg[:])

        def step_mm(layer, t):
            """Recurrent matmuls of `layer` at time t -> gates PSUM tile."""
            if layer == 0:
                wh, gpool, xwd = wh0t, gp0, xw0d
            else:
                wh, gpool, xwd = wh1t, gp1, xw1d

            # xw_t -> [32j+b(+junk), 512]: 32-row loads so every partition
            # of xt is written (rows t*8..t*8+32 wrap into later steps).
            # Loads spread over the Pool/SP DMA queues (SP alone saturates).
            xt = xwp.tile([128, 512], BF16)
            for j in range(4):
                eng = nc.gpsimd if (layer + j) % 2 == 0 else nc.sync
                eng.dma_start(
                    xt[32 * j:32 * (j + 1), :],
                    xwd[t * BC:t * BC + 32, j * 512:(j + 1) * 512])

            ps = gpool.tile([128, 512], F32)
            for k in range(4):
                if t == 0:
                    lh = zer[:]
                elif layer == 0:
                    lh = hist[:, k * plane + (t - 1) * 8:
                              k * plane + (t - 1) * 8 + 32]
                else:
                    sl = (t - 1) % 2
                    lh = ring[:, k * 64 + sl * 32: k * 64 + sl * 32 + 32]
                for j in range(4):
                    nc.tensor.matmul(
                        ps[32 * j:32 * (j + 1), :],
                        lh,
                        wh[k][:, j * 512:(j + 1) * 512],
                        start=(k == 0),
                        stop=(k == 3),
                        tile_position=(0, 32 * j),
                        skip_group_check=True,
                    )
            return ps, xt

        def step_tail(layer, t, ps, xt):
            """Elementwise tail + h transpose + state store for one step."""
            c_t = c0_t if layer == 0 else c1_t

            nc.vector.tensor_add(ps[:], ps[:], xt[:])

            # activations: sigmoid over (i, f, o), tanh over g
            s = sp.tile([128, 512], F32)
            nc.scalar.activation(s[:, 0:384], ps[:, 0:384], AF.Sigmoid)
            nc.scalar.activation(s[:, 384:512], ps[:, 384:512], AF.Tanh)

            # c = f*c + i*g ; h = o * tanh(c)
            ig = hp.tile([128, 128], F32, tag=f"ig{layer}")
            nc.vector.tensor_mul(ig[:], s[:, 0:128], s[:, 384:512])
            fc = hp.tile([128, 128], F32, tag=f"fc{layer}")
            nc.vector.tensor_mul(fc[:], s[:, 128:256], c_t[:])
            nc.vector.tensor_add(c_t[:], fc[:], ig[:])
            th = hp.tile([128, 128], F32, tag=f"th{layer}")
            nc.scalar.activation(th[:], c_t[:], AF.Tanh)
            h = hp.tile([128, 128], F32, tag=f"h{layer}")
            nc.vector.tensor_mul(h[:], s[:, 256:384], th[:])

            # transpose h -> hT, store full 32-wide (8 real + 24 junk)
            pt = tp.tile([128, 128], F32)
            nc.tensor.transpose(pt[:], h[:], ident[:])
            pt4 = pt.rearrange("p (j r) -> p j r", r=32)
            if layer == 0:
                for k in range(4):
                    nc.vector.tensor_copy(
                        hist[:, k * plane + t * 8: k * plane + t * 8 + 32],
                        pt4[:, k, :])
            else:
                sl = t % 2
                for k in range(4):
                    nc.vector.tensor_copy(
                        ring[:, k * 64 + sl * 32: k * 64 + sl * 32 + 32],
                        pt4[:, k, :])
                for j in range(4):
                    eng = nc.gpsimd if j % 2 == 0 else nc.sync
                    eng.dma_start(
                        y[t * BC:(t + 1) * BC, j * 128:(j + 1) * 128],
                        h[32 * j:32 * j + BC, :])

        # ---- main loop ----------------------------------------------------
        # projection work queue: list of (layer, chunk, piece) ready at loop
        # index idx; xw0 chunk c needs nothing -> schedule early; xw1 chunk c
        # needs hist rows of steps [c*CH,(c+1)*CH) -> ready after loop index
        # c*CH + CH - 1 (layer-0 step c*CH+15), consumed from loop index
        # c*CH + LAG.
        work = []           # (ready_idx, layer, chunk, piece)
        for c in range(n_ch):
            for n in range(4):
                work.append((0 if c < 2 else (c - 2) * CH, 0, c, n))
        for c in range(n_ch):
            for n in range(4):
                work.append((c * CH + CH, 1, c, n))
        work.sort(key=lambda w: w[0])
        wq = 0

        total = t_steps + LAG
        for idx in range(total):
            # issue ready projection pieces, max 1 per iteration once warm
            budget = 16 if idx == 0 else 1
            while wq < len(work) and work[wq][0] <= idx and budget > 0:
                _, wlayer, cch, piece = work[wq]
                proj_piece(wlayer, cch, piece)
                wq += 1
                budget -= 1
            # PE order: [L0 mms][L1 mms][L0 transpose][L1 transpose] so each
            # layer's elementwise tail hides under the other layer's stream.
            r0 = step_mm(0, idx) if idx < t_steps else None
            r1 = step_mm(1, idx - LAG) if idx >= LAG else None
            if r0 is not None:
                step_tail(0, idx, *r0)
            if r1 is not None:
                step_tail(1, idx - LAG, *r1)
        assert wq == len(work)

    nc.finalize()
    return nc


def host_inputs(x, date_contexts, w_ih0, w_hh0, w_mh0, b0,
                w_ih1, w_hh1, w_mh1, b1, t_steps=T):
    """Build per-core input maps (layout prep only, no heavy math)."""
    f = lambda a: np.ascontiguousarray(np.asarray(a, np.float32))
    x, ctx = f(x), f(date_contexts)
    w0aug = np.zeros((K0, G), np.float32)
    w0aug[0:I_IN] = f(w_ih0).T
    w0aug[I_IN:I_IN + C] = f(w_mh0).T
    w0aug[I_IN + C] = f(b0)
    w0aug = np.ascontiguousarray(w0aug[:, PERM]).astype(BF16NP)
    wh0p = np.ascontiguousarray(f(w_hh0).T[:, PERM]).astype(BF16NP)
    w1aug = np.zeros((K1, G), np.float32)
    w1aug[0:H] = f(w_ih1).T
    w1aug[H:H + C] = f(w_mh1).T
    w1aug[H + C] = f(b1)
    w1aug = np.ascontiguousarray(w1aug[:, PERM]).astype(BF16NP)
    wh1p = np.ascontiguousarray(f(w_hh1).T[:, PERM]).astype(BF16NP)

    in_maps = []
    for c in range(NCORES):
        bs = slice(c * BC, (c + 1) * BC)
        xTc = np.zeros((K0, t_steps, BC), np.float32)
        xTc[0:I_IN] = np.moveaxis(x[:t_steps, bs, :], 2, 0)
        xTc[I_IN:I_IN + C] = ctx[bs].T[:, None, :]
        xTc[I_IN + C] = 1.0
        a4 = np.zeros((128, 128), np.float32)
        a4[0:C] = np.broadcast_to(
            ctx[bs].T[:, None, :], (C, 16, BC)).reshape(C, 128)
        a4[C] = 1.0
        in_maps.append({
            "xT": np.ascontiguousarray(
                xTc.reshape(K0, t_steps * BC)).astype(BF16NP),
            "w0": w0aug, "wh0": wh0p, "w1": w1aug, "wh1": wh1p,
            "aug4": a4.astype(BF16NP),
        })
    return in_maps


_NC_CACHE = {}


def _get_nc(t_steps=T):
    if t_steps not in _NC_CACHE:
        _NC_CACHE[t_steps] = build_nc(t_steps)
    return _NC_CACHE[t_steps]


def kernel(x, date_contexts, w_ih0, w_hh0, w_mh0, b0,
           w_ih1, w_hh1, w_mh1, b1):
    t_steps = int(os.environ.get("LSTM_T_STEPS", T))
    in_maps = host_inputs(x, date_contexts, w_ih0, w_hh0, w_mh0, b0,
                          w_ih1, w_hh1, w_mh1, b1, t_steps)
    nc = _get_nc(t_steps)
    res = run_bass_kernel_spmd(nc, in_maps, core_ids=list(range(NCORES)))
    out = np.stack(
        [res.results[c]["y"].reshape(t_steps, BC, H) for c in range(NCORES)],
        axis=1,
    )  # [T, NCORES, BC, H]
    return np.ascontiguousarray(
        out.reshape(t_steps, B, H)).astype(np.float32)
